# revision 1
# baseline (speedup 1.0000x reference)
"""3-layer GCN on 8 trn2 NeuronCores.

Strategy (1D graph partitioning, per sharding hint):
 - dst nodes sharded 12500/core; edges partitioned by dst core, sorted by
   (dst-tile-of-128, src-chunk-of-32768), padded per (tile,chunk) to 128.
 - Host precomputes: symmetric GCN norm, per-core int16 gather indices
   (relative to 32K src chunks), and one-hot*norm scatter matrices S.
 - Device per layer: dma_gather of source rows of H=hW from HBM, PE
   matmul segment-sum (S_b^T? no: out[f,dl] += G_b[e,f]^T @ S_b[e,dl]),
   fused bias+residual+ELU on scalar/vector engines, then h@W_next matmul.
 - Host does the inter-layer all-gather (concat of core shards).
"""
import sys

sys.path.insert(0, "/opt/trn_rl_repo")
import numpy as np

import concourse.bacc as bacc
import concourse.bass as bass
import concourse.mybir as mybir
import concourse.tile as tile
from concourse.bass_utils import run_bass_kernel_spmd
from concourse.library_config import mlp

N = 100000
F = 128
NCORE = 8
SH = N // NCORE          # 12500 dst nodes per core
TIL = 128                # dst tile
NT = (SH + TIL - 1) // TIL   # 98
CH = 32768               # src chunk (int16 index limit)
NCH = (N + CH - 1) // CH     # 4
CHUNK_ROWS = [(c * CH, min((c + 1) * CH, N)) for c in range(NCH)]
F32 = mybir.dt.float32
I16 = mybir.dt.int16
ACT = mybir.ActivationFunctionType


def _build_structure(edge_index):
    src = np.asarray(edge_index[0], dtype=np.int64)
    dst = np.asarray(edge_index[1], dtype=np.int64)
    loops = np.arange(N, dtype=np.int64)
    s_all = np.concatenate([src, loops])
    d_all = np.concatenate([dst, loops])
    deg = np.bincount(d_all, minlength=N).astype(np.float64)
    dinv = 1.0 / np.sqrt(deg)
    norm = (dinv[s_all] * dinv[d_all]).astype(np.float32)
    core = d_all // SH
    tloc = (d_all % SH) // TIL
    dl = (d_all % SH) % TIL
    ch = s_all // CH
    key = ((core * NT + tloc) * NCH + ch).astype(np.int64)
    order = np.argsort(key, kind="stable")
    key_s = key[order]
    dl_s = dl[order]
    norm_s = norm[order]
    srel_s = (s_all[order] - ch[order] * CH).astype(np.int16)
    NG = NCORE * NT * NCH
    counts = np.bincount(key_s, minlength=NG)
    starts = np.zeros(NG + 1, np.int64)
    np.cumsum(counts, out=starts[1:])
    rank = np.arange(key_s.size, dtype=np.int64) - starts[key_s]
    cnt = counts.reshape(NCORE, NT, NCH)
    nb = np.maximum(1, -(-cnt.max(axis=0) // 128)).astype(np.int64)  # [NT, NCH]
    boff = np.zeros(NT * NCH + 1, np.int64)
    np.cumsum(nb.reshape(-1), out=boff[1:])
    NBTOT = int(boff[-1])
    EPAD = NBTOT * 128
    tc_idx = key_s % (NT * NCH)
    gpos = boff[tc_idx] * 128 + rank
    core_s = key_s // (NT * NCH)
    Ss, IDXs = [], []
    for c in range(NCORE):
        m = core_s == c
        Sf = np.zeros((EPAD, F), np.float32)
        Sf[gpos[m], dl_s[m]] = norm_s[m]
        S = np.ascontiguousarray(Sf.reshape(NBTOT, 128, F).transpose(1, 0, 2))
        idx = np.zeros((16, NBTOT * 8), np.int16)
        idx[gpos[m] % 16, gpos[m] // 16] = srel_s[m]
        IDXs.append(np.ascontiguousarray(np.tile(idx, (8, 1))))
        Ss.append(S)
    return nb, boff, NBTOT, Ss, IDXs


def _build_progA():
    """H_shard = x_shard @ W (x passed transposed)."""
    nc = bacc.Bacc("TRN2", target_bir_lowering=False)
    xT = nc.dram_tensor("xT", [F, SH], F32, kind="ExternalInput")
    W = nc.dram_tensor("W", [F, F], F32, kind="ExternalInput")
    H = nc.dram_tensor("H", [SH, F], F32, kind="ExternalOutput")
    with tile.TileContext(nc) as tc:
        with (
            tc.tile_pool(name="c0", bufs=1) as cp,
            tc.tile_pool(name="y", bufs=4) as yp,
            tc.tile_pool(name="ps", bufs=4, space=bass.MemorySpace.PSUM) as pp,
        ):
            xT_sb = cp.tile([F, SH], F32)
            nc.sync.dma_start(xT_sb[:], xT[:])
            w_sb = cp.tile([F, F], F32)
            nc.sync.dma_start(w_sb[:], W[:])
            for t in range(NT):
                r0 = t * TIL
                dl = min(TIL, SH - r0)
                ps = pp.tile([TIL, F], F32)
                nc.tensor.matmul(
                    ps[:dl, :], xT_sb[:, r0 : r0 + dl], w_sb[:],
                    start=True, stop=True, skip_group_check=True,
                )
                h = yp.tile([TIL, F], F32)
                nc.vector.tensor_copy(h[:dl, :], ps[:dl, :])
                nc.sync.dma_start(H[r0 : r0 + dl, :], h[:dl, :])
    nc.compile()
    return nc


def _build_progB(nb, boff, NBTOT):
    """One GCN layer: h' = ELU(A_hat @ Hf + b + res); Hn = h' @ Wn."""
    nc = bacc.Bacc("TRN2", target_bir_lowering=False)
    Hf = nc.dram_tensor("Hf", [N, F], F32, kind="ExternalInput")
    S = nc.dram_tensor("S", [128, NBTOT, F], F32, kind="ExternalInput")
    IDX = nc.dram_tensor("IDX", [128, NBTOT * 8], I16, kind="ExternalInput")
    RT = nc.dram_tensor("RT", [F, SH], F32, kind="ExternalInput")
    BC = nc.dram_tensor("BC", [F, 1], F32, kind="ExternalInput")
    BN = nc.dram_tensor("BN", [F, 1], F32, kind="ExternalInput")
    WN = nc.dram_tensor("WN", [F, F], F32, kind="ExternalInput")
    HT = nc.dram_tensor("HT", [F, SH], F32, kind="ExternalOutput")
    HN = nc.dram_tensor("HN", [SH, F], F32, kind="ExternalOutput")
    with tile.TileContext(nc) as tc:
        with (
            tc.tile_pool(name="c0", bufs=1) as cp,
            tc.tile_pool(name="sp", bufs=2) as sp,
            tc.tile_pool(name="gp", bufs=3) as gp,
            tc.tile_pool(name="yp", bufs=8) as yp,
            tc.tile_pool(name="ps", bufs=4, space=bass.MemorySpace.PSUM) as pp,
        ):
            nc.gpsimd.load_library(mlp)
            idx_sb = cp.tile([128, NBTOT * 8], I16)
            nc.sync.dma_start(idx_sb[:], IDX[:])
            wn_sb = cp.tile([F, F], F32)
            nc.sync.dma_start(wn_sb[:], WN[:])
            bc_sb = cp.tile([F, 1], F32)
            nc.sync.dma_start(bc_sb[:], BC[:])
            bn_sb = cp.tile([F, 1], F32)
            nc.sync.dma_start(bn_sb[:], BN[:])
            for t in range(NT):
                r0 = t * TIL
                dl = min(TIL, SH - r0)
                b0 = int(boff[t * NCH])
                nbt = int(boff[(t + 1) * NCH] - b0)
                s_sb = sp.tile([128, nbt, F], F32)
                nc.sync.dma_start(s_sb[:], S[:, b0 : b0 + nbt, :])
                ps = pp.tile([F, TIL], F32)
                mm = 0
                for c in range(NCH):
                    nbc = int(nb[t][c])
                    bo = int(boff[t * NCH + c]) - b0
                    col0 = int(boff[t * NCH + c]) * 8
                    g_sb = gp.tile([128, nbc, F], F32)
                    nc.gpsimd.dma_gather(
                        g_sb[:],
                        Hf[CHUNK_ROWS[c][0] : CHUNK_ROWS[c][1], :],
                        idx_sb[:, col0 : col0 + nbc * 8],
                        nbc * 128, nbc * 128, F,
                    )
                    for j in range(nbc):
                        nc.tensor.matmul(
                            ps[:], g_sb[:, j, :], s_sb[:, bo + j, :],
                            start=(mm == 0), stop=(mm == nbt - 1),
                            skip_group_check=True,
                        )
                        mm += 1
                res = yp.tile([F, TIL], F32)
                nc.sync.dma_start(res[:, :dl], RT[:, r0 : r0 + dl])
                y1 = yp.tile([F, TIL], F32)
                nc.vector.tensor_tensor(
                    y1[:, :dl], ps[:, :dl], res[:, :dl], mybir.AluOpType.add
                )
                a = yp.tile([F, TIL], F32)
                nc.scalar.activation(a[:, :dl], y1[:, :dl], ACT.Relu, bias=bc_sb[:, 0:1])
                ng = yp.tile([F, TIL], F32)
                nc.scalar.activation(
                    ng[:, :dl], y1[:, :dl], ACT.Relu, bias=bn_sb[:, 0:1], scale=-1.0
                )
                e = yp.tile([F, TIL], F32)
                nc.scalar.activation(e[:, :dl], ng[:, :dl], ACT.Exp, scale=-1.0)
                em = yp.tile([F, TIL], F32)
                nc.vector.tensor_scalar_add(em[:, :dl], e[:, :dl], -1.0)
                h = yp.tile([F, TIL], F32)
                nc.vector.tensor_tensor(
                    h[:, :dl], a[:, :dl], em[:, :dl], mybir.AluOpType.add
                )
                nc.sync.dma_start(HT[:, r0 : r0 + dl], h[:, :dl])
                ps2 = pp.tile([TIL, F], F32)
                nc.tensor.matmul(
                    ps2[:dl, :], h[:, :dl], wn_sb[:],
                    start=True, stop=True, skip_group_check=True,
                )
                hn = yp.tile([TIL, F], F32)
                nc.vector.tensor_copy(hn[:dl, :], ps2[:dl, :])
                nc.sync.dma_start(HN[r0 : r0 + dl, :], hn[:dl, :])
    nc.compile()
    return nc


LAUNCH_TIMES = []


def kernel(x, edge_index, W0, b0, W1, b1, W2, b2, _trace=False):
    import time as _time
    x = np.ascontiguousarray(np.asarray(x, dtype=np.float32))
    W0 = np.asarray(W0, np.float32)
    W1 = np.asarray(W1, np.float32)
    W2 = np.asarray(W2, np.float32)
    nb, boff, NBTOT, Ss, IDXs = _build_structure(np.asarray(edge_index))
    ncA = _build_progA()
    ncB = _build_progB(nb, boff, NBTOT)
    cores = list(range(NCORE))

    in_maps = [
        {"xT": np.ascontiguousarray(x[c * SH : (c + 1) * SH].T), "W": W0}
        for c in range(NCORE)
    ]
    resA = run_bass_kernel_spmd(ncA, in_maps, cores)
    H = np.ascontiguousarray(
        np.concatenate([resA.results[c]["H"] for c in range(NCORE)], axis=0)
    )

    hprevT = [np.zeros((F, SH), np.float32) for _ in range(NCORE)]
    Wnext = [W1, W2, np.zeros((F, F), np.float32)]
    bs = [np.asarray(b0, np.float32), np.asarray(b1, np.float32), np.asarray(b2, np.float32)]
    traces = []
    for l in range(3):
        bc = np.ascontiguousarray(bs[l].reshape(F, 1))
        in_maps = [
            {
                "Hf": H, "S": Ss[c], "IDX": IDXs[c], "RT": hprevT[c],
                "BC": bc, "BN": np.ascontiguousarray(-bc), "WN": Wnext[l],
            }
            for c in range(NCORE)
        ]
        _t0 = _time.time()
        resB = run_bass_kernel_spmd(ncB, in_maps, cores)
        LAUNCH_TIMES.append(_time.time() - _t0)
        traces.append(resB)
        hprevT = [resB.results[c]["HT"] for c in range(NCORE)]
        H = np.ascontiguousarray(
            np.concatenate([resB.results[c]["HN"] for c in range(NCORE)], axis=0)
        )
    out = np.concatenate([np.ascontiguousarray(hT.T) for hT in hprevT], axis=0)
    if _trace:
        return out.astype(np.float32), traces
    return out.astype(np.float32)

